# revision 38
# baseline (speedup 1.0000x reference)
"""Trainium2 Bass kernel for nn_Actor_12446815223911 (dense_mlp).

Network (per row, batch B=262144):
  x [126] -> 14 groups of 9 -> shared (9->16 relu, 16->4 relu) -> concat 56
  combined = [56 | sale] (57) -> relu(57->128) -> 128->30
  mean = tanh(out[:15]), std = exp(out[15:])

Strategy: pure data parallel over 8 cores (32768 rows each), feature-major
layout (features on partitions, batch on free dim). Block-diagonal weights
collapse the grouped layers; zero weight rows/cols neutralize padding and
initialize PSUM gaps. Elementwise ops are the bottleneck (ACT+DVE are the
only PSUM readers, ~250ns fixed overhead per op), so chunks are processed
in superchunks of 2x512 rows with wide (1024-col) relu ops and the small
layers packed multiple-chunks-per-PSUM-bank. Host pre-transposes inputs /
post-transposes outputs.
"""

import numpy as np
import ml_dtypes

B = 262144
N_CORES = 8
BPC = B // N_CORES  # 32768 rows per core
CH = 512            # rows per matmul chunk (one PSUM bank fp32)
SC = 2 * CH         # superchunk: elementwise ops are 1024 wide
XT_COLS = 8192      # rows per input DMA tile

BF16 = ml_dtypes.bfloat16

_CACHE = {}

PROFILE = False
LAST_EXEC_NS = None
LAST_TRACE_DIR = None


def _build_nc(bpc):
    """Build the single-core Bass graph (SPMD: all 8 cores run this)."""
    import concourse.bacc as bacc
    import concourse.mybir as mybir
    import concourse.tile as tile

    f32 = mybir.dt.float32
    bf16 = mybir.dt.bfloat16
    AF = mybir.ActivationFunctionType
    ALU = mybir.AluOpType

    n_chunks = bpc // CH
    xt_cols = min(XT_COLS, bpc)
    ch_per_xt = xt_cols // CH

    nc = bacc.Bacc("TRN2", target_bir_lowering=False, debug=False)

    xt_d = nc.declare_dram_parameter("xt", [128, bpc], bf16, isOutput=False)
    sale_d = nc.declare_dram_parameter("sale", [1, bpc], bf16, isOutput=False)
    w1_d = nc.declare_dram_parameter("w1t", [128, 112], bf16, isOutput=False)
    w2_d = nc.declare_dram_parameter("w2t", [112, 64], bf16, isOutput=False)
    w3a_d = nc.declare_dram_parameter("w3a", [128, 128], bf16, isOutput=False)
    w3b_d = nc.declare_dram_parameter("w3b", [128, 128], bf16, isOutput=False)
    w4_d = nc.declare_dram_parameter("w4t", [128, 32], bf16, isOutput=False)
    b1_d = nc.declare_dram_parameter("b1r", [112, 1], f32, isOutput=False)
    b2_d = nc.declare_dram_parameter("b2r", [125, 1], f32, isOutput=False)
    b3_d = nc.declare_dram_parameter("b3r", [128, 1], f32, isOutput=False)
    b4_d = nc.declare_dram_parameter("b4r", [128, 1], f32, isOutput=False)
    mean_d = nc.declare_dram_parameter("mean_t", [15, bpc], f32, isOutput=True)
    std_d = nc.declare_dram_parameter("std_t", [15, bpc], f32, isOutput=True)

    with tile.TileContext(nc) as tc:
        with (
            tc.tile_pool(name="consts", bufs=1) as consts,
            tc.tile_pool(name="xt", bufs=3) as xt_pool,
            tc.tile_pool(name="work", bufs=4) as work,
            tc.tile_pool(name="outs", bufs=3) as outs,
            # PSUM budget (8 banks): h 4 + s 1 + z 2 + out 1
            tc.tile_pool(name="ps_h", bufs=4, space="PSUM") as ps_h,
            tc.tile_pool(name="ps_s", bufs=1, space="PSUM") as ps_s,
            tc.tile_pool(name="ps_z", bufs=2, space="PSUM") as ps_z,
            tc.tile_pool(name="ps_o", bufs=1, space="PSUM") as ps_o,
        ):
            w1 = consts.tile([128, 112], bf16)
            w2 = consts.tile([112, 64], bf16)
            w3a = consts.tile([128, 128], bf16)
            w3b = consts.tile([128, 128], bf16)
            w4 = consts.tile([128, 32], bf16)
            b1 = consts.tile([112, 1], f32)
            b2 = consts.tile([125, 1], f32)
            b3 = consts.tile([128, 1], f32)
            b4 = consts.tile([128, 1], f32)
            nc.sync.dma_start(w1[:], w1_d[:])
            nc.sync.dma_start(w2[:], w2_d[:])
            nc.sync.dma_start(w3a[:], w3a_d[:])
            nc.sync.dma_start(w3b[:], w3b_d[:])
            nc.sync.dma_start(w4[:], w4_d[:])
            nc.sync.dma_start(b1[:], b1_d[:])
            nc.sync.dma_start(b2[:], b2_d[:])
            nc.sync.dma_start(b3[:], b3_d[:])
            nc.sync.dma_start(b4[:], b4_d[:])


            out_bank = None
            s_ps = None
            comb = None
            for c in range(n_chunks):
                po = c % 2           # position within the 2-chunk s-pack
                q = c % 4            # out-bank slot
                g = c // 4           # output group (4 chunks)

                xq = c % ch_per_xt
                if c == 0:
                    xt = xt_pool.tile([128, xt_cols], bf16, tag="xt")
                    for sl in range(0, xt_cols, 1024):
                        nc.sync.dma_start(
                            xt[:, sl : sl + 1024],
                            xt_d[:, sl : sl + 1024])
                elif xq == 0:
                    xt, xt_next = xt_next, None
                # prefetch the next tile's slices one per chunk, starting
                # mid-tile, so the input stream is smooth instead of bursty
                half = ch_per_xt // 2
                if xq >= half and c - xq + ch_per_xt < n_chunks:
                    if xq == half:
                        xt_next = xt_pool.tile([128, xt_cols], bf16, tag="xt")
                    base = (c - xq + ch_per_xt) * CH
                    sl = (xq - half) * (xt_cols // half)
                    w = xt_cols // half
                    nc.sync.dma_start(xt_next[:, sl : sl + w],
                                      xt_d[:, base + sl : base + sl + w])
                xs = xt[:, xq * CH : (xq + 1) * CH]

                # L1: concurrent row-tiled pair -> h halves [112, CH]
                hA_ps = ps_h.tile([112, CH], f32, tag="h")
                hB_ps = ps_h.tile([112, CH], f32, tag="h")
                nc.tensor.matmul(hA_ps[:], w1[0:64, :], xs[0:64, :],
                                 start=True, stop=True)
                nc.tensor.matmul(hB_ps[:], w1[64:128, :], xs[64:128, :],
                                 start=True, stop=True)

                hA = work.tile([112, CH], bf16, tag="hA")
                hB = work.tile([112, CH], bf16, tag="hB")
                nc.scalar.activation(hA[:], hA_ps[:], AF.Relu, bias=b1[:])
                nc.vector.tensor_scalar(hB[:], hB_ps[:], b1[:], 0.0,
                                        ALU.add, ALU.max)

                # L2: col-tiled pair into the shared 2-chunk s bank
                # partitions: 0:32 c0-A | 32:64 c0-B | 64:96 c1-A | 96:128 c1-B
                if po == 0:
                    s_ps = ps_s.tile([128, CH], f32, tag="s")
                    comb = work.tile([126, CH], bf16, tag="comb")
                    # sale of chunk c+1 lives outside the relu range
                    nc.gpsimd.dma_start(comb[125:126, :],
                                        sale_d[0:1, (c + 1) * CH : (c + 2) * CH])
                # L2: each chunk's col-tiled pair issues as soon as its relu
                # is done (shorter chain than a deferred 4-wide quad)
                nc.tensor.matmul(s_ps[64 * po : 64 * po + 32, :], w2[:, 0:32],
                                 hA[:], start=True, stop=True,
                                 tile_position=(0, 64 * po))
                nc.tensor.matmul(s_ps[64 * po + 32 : 64 * po + 64, :],
                                 w2[:, 32:64], hB[:], start=True, stop=True,
                                 tile_position=(0, 64 * po + 32))
                if po == 1:
                    # one relu covers both chunks of the pack
                    nc.scalar.activation(comb[0:125, :], s_ps[0:125, :],
                                         AF.Relu, bias=b2[:])
                    # sale of chunk c-1 goes INSIDE the c0 window (partition
                    # 60, a zero-pad lane) so L3-even's K window is [0:61] and
                    # the two L3s pair on disjoint row strips. Overwrites the
                    # relu's zero-pad output, so ordered after it.
                    nc.gpsimd.dma_start(comb[60:61, :],
                                        sale_d[0:1, (c - 1) * CH : c * CH])
                    zp0 = ps_z.tile([128, CH], f32, tag="z", name="zp0")
                    zp1 = ps_z.tile([128, CH], f32, tag="z", name="zp1")
                    nc.tensor.matmul(zp0[:], w3a[0:61, :], comb[0:61, :],
                                     start=True, stop=True)
                    nc.tensor.matmul(zp1[:], w3b[64:126, :], comb[64:126, :],
                                     start=True, stop=True)
                    z = work.tile([128, 2 * CH], bf16, tag="z")
                    nc.vector.tensor_scalar(z[:, 0:CH], zp0[:], b3[:], 0.0,
                                            ALU.add, ALU.max)
                    nc.vector.tensor_scalar(z[:, CH:2 * CH], zp1[:], b3[:],
                                            0.0, ALU.add, ALU.max)
                    qq = (c - 1) % 4
                    if qq == 0:
                        out_bank = ps_o.tile([128, CH], f32, tag="o")
                    nc.tensor.matmul(out_bank[32 * qq : 32 * qq + 32, :],
                                     w4[:], z[:, 0:CH], start=True, stop=True,
                                     tile_position=(0, 32 * qq))
                    nc.tensor.matmul(out_bank[32 * qq + 32 : 32 * qq + 64, :],
                                     w4[:], z[:, CH:2 * CH], start=True,
                                     stop=True, tile_position=(0, 32 * qq + 32))

                if q == 3:
                    gg = g % 2       # position within the 2-group staging tile
                    if gg == 0:
                        t1 = outs.tile([128, 2 * CH], f32, tag="t1")
                        t2 = outs.tile([128, 2 * CH], f32, tag="t2")
                    nc.scalar.activation(t1[:, gg * CH : (gg + 1) * CH],
                                         out_bank[:], AF.Tanh, bias=b4[:])
                    nc.scalar.activation(t2[:, gg * CH : (gg + 1) * CH],
                                         out_bank[:], AF.Exp, bias=b4[:])
                    if gg == 1:
                        # 8 chunks done: chunk (g-1+ggi)*4+k row j lives at
                        # t[32k+j, ggi*CH:...]. One DMA per k per output.
                        base = (g - 1) * 4
                        for k in range(4):
                            dst_m = mean_d[:, (base + k) * CH :].rearrange(
                                "j (gg n) -> j gg n", n=CH)[:, 0:5:4, :]
                            dst_s = std_d[:, (base + k) * CH :].rearrange(
                                "j (gg n) -> j gg n", n=CH)[:, 0:5:4, :]
                            src_m = t1[32 * k : 32 * k + 15, :].rearrange(
                                "j (gg n) -> j gg n", n=CH)
                            src_s = t2[32 * k + 15 : 32 * k + 30, :].rearrange(
                                "j (gg n) -> j gg n", n=CH)
                            nc.sync.dma_start(dst_m, src_m)
                            nc.sync.dma_start(dst_s, src_s)

    nc.finalize()
    return nc


def _pack_consts(W1, b1, W2, b2, W3, b3, W4, b4):
    """Host-side weight packing into the on-chip layouts."""
    f32 = np.float32
    W1 = np.asarray(W1, f32); W2 = np.asarray(W2, f32)
    W3 = np.asarray(W3, f32); W4 = np.asarray(W4, f32)
    b1 = np.asarray(b1, f32); b2 = np.asarray(b2, f32)
    b3 = np.asarray(b3, f32); b4 = np.asarray(b4, f32)

    w1t = np.zeros((128, 112), f32)
    for half, base in ((0, 0), (1, 64)):
        for i in range(7):
            w1t[base + 9 * i: base + 9 * i + 9, 16 * i: 16 * i + 16] = W1
    # w2t [112, 64]: cols 0:28 groups 0-6 block-diag, cols 32:60 groups 7-13
    w2t = np.zeros((112, 64), f32)
    for half in range(2):
        for i in range(7):
            w2t[16 * i: 16 * i + 16, 32 * half + 4 * i: 32 * half + 4 * i + 4] = W2
    # s-bank partition layout: 0:28 c0·sA, 32:60 c0·sB, 64:92 c1·sA,
    # 96:124 c1·sB, 124 sale0, 125 sale1 (sales only in the comb SBUF tile)
    w3a = np.zeros((128, 128), f32)   # chunk 0: K window [0:61]
    w3a[0:28] = W3[0:28]
    w3a[32:60] = W3[28:56]
    w3a[60] = W3[56]
    w3b = np.zeros((128, 128), f32)   # chunk 1: K window [64:126]
    w3b[64:92] = W3[0:28]
    w3b[96:124] = W3[28:56]
    w3b[125] = W3[56]
    w4t = np.zeros((128, 32), f32)
    w4t[:, 0:30] = W4

    b1r = np.tile(b1, 7)[:, None]                      # [112, 1]
    b2r = np.zeros((125, 1), f32)
    for base in (0, 32, 64, 96):
        b2r[base: base + 28, 0] = np.tile(b2, 7)
    b3r = b3[:, None]                                  # [128, 1]
    b4r = np.zeros((128, 1), f32)
    for k in range(4):
        b4r[32 * k: 32 * k + 30, 0] = b4
    return {
        "w1t": w1t.astype(BF16), "w2t": w2t.astype(BF16),
        "w3a": w3a.astype(BF16), "w3b": w3b.astype(BF16),
        "w4t": w4t.astype(BF16),
        "b1r": b1r, "b2r": b2r, "b3r": b3r, "b4r": b4r,
    }


def _pack_x(features_2):
    """[B, 126] f32 -> padded transposed [128, B] bf16."""
    Bn = features_2.shape[0]
    xt = np.zeros((128, Bn), dtype=BF16)
    xf = np.asarray(features_2, np.float32)
    xt[0:63] = xf[:, 0:63].T.astype(BF16)
    xt[64:127] = xf[:, 63:126].T.astype(BF16)
    return xt


def kernel(features_2, sale_predictions, W1, b1, W2, b2, W3, b3, W4, b4):
    global LAST_EXEC_NS, LAST_TRACE_DIR
    from concourse.bass_utils import run_bass_kernel_spmd

    Bn = features_2.shape[0]
    assert Bn == B and Bn % N_CORES == 0
    bpc = Bn // N_CORES

    if bpc not in _CACHE:
        _CACHE[bpc] = _build_nc(bpc)
    nc = _CACHE[bpc]

    consts = _pack_consts(W1, b1, W2, b2, W3, b3, W4, b4)
    xt = _pack_x(features_2)
    sale = np.asarray(sale_predictions, np.float32)[:, 0].astype(BF16)[None, :]

    in_maps = []
    for i in range(N_CORES):
        m = dict(consts)
        m["xt"] = np.ascontiguousarray(xt[:, i * bpc: (i + 1) * bpc])
        m["sale"] = np.ascontiguousarray(sale[:, i * bpc: (i + 1) * bpc])
        in_maps.append(m)

    res = run_bass_kernel_spmd(
        nc, in_maps, core_ids=list(range(N_CORES)), trace=PROFILE
    )
    LAST_EXEC_NS = res.exec_time_ns
    LAST_TRACE_DIR = getattr(res, "trace_dir", None)

    mean_t = np.concatenate([res.results[i]["mean_t"] for i in range(N_CORES)],
                            axis=1)
    std_t = np.concatenate([res.results[i]["std_t"] for i in range(N_CORES)],
                           axis=1)
    action_mean = np.ascontiguousarray(mean_t.T, dtype=np.float32)
    action_std = np.ascontiguousarray(std_t.T, dtype=np.float32)
    return (action_mean, action_std)
